# revision 2
# baseline (speedup 1.0000x reference)
"""Content-based (Bahdanau-style) attention kernel for Trainium2.

Computes, per batch b:
    e      = tanh(keys @ W_s.T + q[b] @ W_h.T + b)     # [S, H]
    energy = e @ v                                      # [S]
    w      = softmax(energy)                            # [S]
    ctx    = w @ keys                                   # [H]

Full shapes: keys [32, 4096, 512], q [1, 32, 512], W* [512, 512].
Sharding: data-parallel over the batch dim -> 4 batches per core on 8
NeuronCores, weights replicated, no collectives. Output gathered on host.

Per-core pipeline (fp32 end to end, PE streams 1 col/cycle regardless of
dtype so fp32 costs the same as bf16 here):
  - W_s.T / W_h.T built once via PE transposes (fp32 has no DMA transpose).
  - q @ W_h.T + b computed once into per-(batch, o-chunk) bias columns.
  - Per 512-token block: keys tiles transposed on PE, main matmul
    accumulates pre.T [o, t] in PSUM, ScalarE applies tanh with the fused
    per-partition bias, PE dots with v for the energies, ScalarE Exp with
    fused denominator accumulation, PE re-transposes the weights to a
    column and accumulates the unnormalized context over the whole batch
    in PSUM. One division at batch end.
Softmax max-subtraction is skipped deliberately: |energy| <= sum|v| ~ 20,
exp() cannot overflow fp32.
"""

import numpy as np
from contextlib import ExitStack

import concourse.bass as bass
import concourse.tile as tile
from concourse import mybir
from concourse.bass_utils import run_bass_kernel_spmd
from concourse.masks import make_identity

H = 512
S = 4096
B = 32
N_CORES = 8
LOCAL_B = B // N_CORES
FP = mybir.dt.float32
TBLK = 512  # tokens per inner block

MAX_WAITS = 1


def split_sync_waits(nc):
    """This container's walrus rejects >1 sem-wait per instruction in some
    encodings; split overflow waits onto carrier nops placed just before
    the offending instruction (same engine, so ordering is preserved)."""
    n_split = 0
    for f in nc.m.functions:
        for bb in f.blocks:
            snapshot = list(bb.instructions)
            inserts = []
            for idx, ins in enumerate(snapshot):
                w = ins.sync_info.on_wait if ins.sync_info else None
                if w and len(w) > MAX_WAITS:
                    chunks = [w[i:i + MAX_WAITS] for i in range(0, len(w), MAX_WAITS)]
                    ins.sync_info.on_wait = chunks[-1]
                    nops = []
                    for j, ch in enumerate(chunks[:-1]):
                        nop = mybir.InstNoOp(
                            name=f"waitsplit-{ins.name}-{j}", ins=[], outs=[])
                        nop.engine = ins.engine
                        nop.sync_info = mybir.SyncInfo(on_wait=ch, on_update=[])
                        nops.append(nop)
                    inserts.append((idx, nops))
                    n_split += 1
            for idx, nops in reversed(inserts):
                for nop in reversed(nops):
                    bb.instructions.insert(idx, nop)
    return n_split


def build(local_b=LOCAL_B, s=S, repeat=1, split_waits=True):
    """Build the per-core Bass program. `repeat` re-runs the whole body
    (identical outputs) for wall-clock differencing in test harnesses."""
    nc = bass.Bass()
    keys_d = nc.declare_dram_parameter("keys", [local_b * s, H], FP, isOutput=False)
    q_d = nc.declare_dram_parameter("q", [local_b, H], FP, isOutput=False)
    wh_d = nc.declare_dram_parameter("W_h", [H, H], FP, isOutput=False)
    ws_d = nc.declare_dram_parameter("W_s", [H, H], FP, isOutput=False)
    v_d = nc.declare_dram_parameter("v", [H], FP, isOutput=False)
    b_d = nc.declare_dram_parameter("b", [H], FP, isOutput=False)
    out_d = nc.declare_dram_parameter("out", [local_b, H], FP, isOutput=True)

    n_tblk = s // TBLK
    LB = local_b

    with ExitStack() as ctx:
        tc = ctx.enter_context(tile.TileContext(nc))
        const_pool = ctx.enter_context(tc.tile_pool(name="const", bufs=1))
        kn_pool = ctx.enter_context(tc.tile_pool(name="kn", bufs=8))
        kt_pool = ctx.enter_context(tc.tile_pool(name="kt", bufs=8))
        et_pool = ctx.enter_context(tc.tile_pool(name="et", bufs=8))
        small_pool = ctx.enter_context(tc.tile_pool(name="small", bufs=4))
        psum_tp = ctx.enter_context(tc.tile_pool(name="ptp", bufs=2, space="PSUM"))
        psum_pre = ctx.enter_context(tc.tile_pool(name="ppre", bufs=2, space="PSUM"))
        psum_misc = ctx.enter_context(tc.tile_pool(name="pmisc", bufs=1, space="PSUM"))
        psum_ctxp = ctx.enter_context(tc.tile_pool(name="pctx", bufs=1, space="PSUM"))

        ident = const_pool.tile([128, 128], FP)
        make_identity(nc, ident)
        ones_row = const_pool.tile([1, 128], FP)
        nc.vector.memset(ones_row, 1.0)

        def load_transposed(w_dram, tag):
            """w_dram [o, i] row-major -> list of 4 SBUF tiles wT[ic] [128 i, 512 o]."""
            nat = []
            for oc in range(4):
                t = const_pool.tile([128, H], FP, tag=f"{tag}nat{oc}")
                nc.sync.dma_start(out=t, in_=w_dram[oc * 128:(oc + 1) * 128, :])
                nat.append(t)
            wT = []
            for ic in range(4):
                tt = const_pool.tile([128, H], FP, tag=f"{tag}T{ic}")
                pt = psum_tp.tile([128, TBLK], FP, tag="tp")
                for oc in range(4):
                    nc.tensor.transpose(
                        pt[:, oc * 128:(oc + 1) * 128],
                        nat[oc][:, ic * 128:(ic + 1) * 128], ident)
                nc.vector.tensor_copy(tt[:, :H], pt[:, :H])
                wT.append(tt)
            return wT

        wsT = load_transposed(ws_d, "ws")
        whT = load_transposed(wh_d, "wh")

        q_sb = const_pool.tile([LB, H], FP)
        nc.sync.dma_start(out=q_sb, in_=q_d[:, :])
        b_sb = const_pool.tile([1, H], FP)
        nc.sync.dma_start(out=b_sb, in_=b_d[:].rearrange("(o h) -> o h", o=1))
        v_sb = const_pool.tile([128, 4], FP)
        nc.sync.dma_start(out=v_sb, in_=v_d[:].rearrange("(c p) -> p c", p=128))

        # qT[ic] columns: [128 i, LB]
        qT = const_pool.tile([128, 4 * LB], FP)
        for ic in range(4):
            pt = psum_tp.tile([128, TBLK], FP, tag="tp")
            nc.tensor.transpose(pt[:, :LB], q_sb[:, ic * 128:(ic + 1) * 128],
                                ident[:LB, :LB])
            nc.vector.tensor_copy(qT[:, ic * LB:(ic + 1) * LB], pt[:, :LB])

        # qwh[b, o] = q[b] @ W_h.T + b  -> transposed to per-partition bias cols
        pq = psum_pre.tile([128, TBLK], FP, tag="pre")
        for ic in range(4):
            nc.tensor.matmul(pq[:LB, :H], lhsT=qT[:, ic * LB:(ic + 1) * LB],
                             rhs=whT[ic], start=(ic == 0), stop=False)
        nc.tensor.matmul(pq[:LB, :H], lhsT=ones_row[:, :LB], rhs=b_sb,
                         start=False, stop=True)
        qwh_sb = const_pool.tile([LB, H], FP)
        nc.scalar.copy(qwh_sb, pq[:LB, :H])
        qwhbT = const_pool.tile([128, 4 * LB], FP)
        for oc in range(4):
            pt = psum_tp.tile([128, TBLK], FP, tag="tp")
            nc.tensor.transpose(pt[:, :LB], qwh_sb[:, oc * 128:(oc + 1) * 128],
                                ident[:LB, :LB])
            nc.vector.tensor_copy(qwhbT[:, oc * LB:(oc + 1) * LB], pt[:, :LB])

        for rep in range(repeat):
            for lb in range(LB):
                pctx = psum_ctxp.tile([1, H], FP, tag="ctx")
                denom = small_pool.tile([1, n_tblk], FP, tag="denom")
                for tb in range(n_tblk):
                    base = lb * s + tb * TBLK
                    kn = []
                    for t4 in range(4):
                        t = kn_pool.tile([128, H], FP, tag="kn")
                        nc.sync.dma_start(
                            out=t,
                            in_=keys_d[base + t4 * 128: base + (t4 + 1) * 128, :])
                        kn.append(t)
                    kts = []
                    for ic in range(4):
                        pt = psum_tp.tile([128, TBLK], FP, tag="tp")
                        for t4 in range(4):
                            nc.tensor.transpose(
                                pt[:, t4 * 128:(t4 + 1) * 128],
                                kn[t4][:, ic * 128:(ic + 1) * 128], ident)
                        kt = kt_pool.tile([128, TBLK], FP, tag="kt")
                        nc.vector.tensor_copy(kt, pt)
                        kts.append(kt)
                    pe_energy = psum_misc.tile([1, TBLK], FP, tag="energy")
                    for oc in range(4):
                        ppre = psum_pre.tile([128, TBLK], FP, tag="pre")
                        for ic in range(4):
                            nc.tensor.matmul(
                                ppre, lhsT=wsT[ic][:, oc * 128:(oc + 1) * 128],
                                rhs=kts[ic], start=(ic == 0), stop=(ic == 3))
                        et = et_pool.tile([128, TBLK], FP, tag="et")
                        nc.scalar.activation(
                            et, ppre, mybir.ActivationFunctionType.Tanh,
                            bias=qwhbT[:, oc * LB + lb: oc * LB + lb + 1],
                            scale=1.0)
                        nc.tensor.matmul(pe_energy, lhsT=v_sb[:, oc:oc + 1],
                                         rhs=et, start=(oc == 0), stop=(oc == 3))
                    w_row = small_pool.tile([1, TBLK], FP, tag="wrow")
                    nc.scalar.activation(w_row, pe_energy,
                                         mybir.ActivationFunctionType.Exp,
                                         accum_out=denom[:, tb:tb + 1])
                    pwT = psum_misc.tile([128, 4], FP, tag="wT")
                    for t4 in range(4):
                        nc.tensor.transpose(pwT[:, t4:t4 + 1],
                                            w_row[:, t4 * 128:(t4 + 1) * 128],
                                            ident[:1, :1])
                    w_col = small_pool.tile([128, 4], FP, tag="wcol")
                    nc.vector.tensor_copy(w_col, pwT)
                    for t4 in range(4):
                        nc.tensor.matmul(
                            pctx, lhsT=w_col[:, t4:t4 + 1], rhs=kn[t4],
                            start=(tb == 0 and t4 == 0),
                            stop=(tb == n_tblk - 1 and t4 == 3))
                dsum = small_pool.tile([1, 1], FP, tag="dsum")
                nc.vector.tensor_reduce(dsum, denom, axis=mybir.AxisListType.X,
                                        op=mybir.AluOpType.add)
                rec = small_pool.tile([1, 1], FP, tag="rec")
                nc.vector.reciprocal(rec, dsum)
                ctx_row = small_pool.tile([1, H], FP, tag="ctxrow")
                nc.vector.tensor_scalar_mul(ctx_row, pctx, rec)
                nc.sync.dma_start(out=out_d[lb:lb + 1, :], in_=ctx_row)

    if split_waits:
        split_sync_waits(nc)
    return nc


_NC_CACHE = {}


def _get_nc(repeat=1):
    if repeat not in _NC_CACHE:
        _NC_CACHE[repeat] = build(repeat=repeat)
    return _NC_CACHE[repeat]


def kernel(encoder_outputs, decoder_h_t, W_h, W_s, v, b):
    keys = np.ascontiguousarray(np.asarray(encoder_outputs, dtype=np.float32))
    q = np.ascontiguousarray(np.asarray(decoder_h_t, dtype=np.float32))[0]  # [B, H]
    W_h = np.ascontiguousarray(np.asarray(W_h, dtype=np.float32))
    W_s = np.ascontiguousarray(np.asarray(W_s, dtype=np.float32))
    v = np.ascontiguousarray(np.asarray(v, dtype=np.float32))
    b = np.ascontiguousarray(np.asarray(b, dtype=np.float32))

    nc = _get_nc()
    in_maps = []
    for c in range(N_CORES):
        lo, hi = c * LOCAL_B, (c + 1) * LOCAL_B
        in_maps.append({
            "keys": keys[lo:hi].reshape(LOCAL_B * S, H),
            "q": q[lo:hi],
            "W_h": W_h,
            "W_s": W_s,
            "v": v,
            "b": b,
        })
    res = run_bass_kernel_spmd(nc, in_maps, core_ids=list(range(N_CORES)))
    out = np.concatenate([res.results[c]["out"] for c in range(N_CORES)], axis=0)
    return out.reshape(B, 1, H).astype(np.float32)
